# revision 35
# baseline (speedup 1.0000x reference)
"""Trainium2 Bass kernel for a causal single-head attention block.

Problem: y = softmax(mask(Q K^T / sqrt(H))) V with
  x  [B=4, T=4096, C=1024] f32,  Wq/Wk/Wv [C, H=64] f32.

Sharding (8 NeuronCores): data-parallel over B across core pairs;
within a pair, T is split by interleaved 512-row tiles (rank r owns
global q-tiles {2s+r}).  Each core projects K/V for its own 2048 rows,
the pair exchanges them via AllGather, and each core runs a
flash-attention style pair-of-kc outer loop over its own query rows.
The graph is identical on all 8 cores (SPMD); all rank-dependent
causality is delivered via input *data* (per-rank mask sheets).

Structure (v5):
 - xT streams in per-slot over the sync/scalar DMA queues (HBM-bound,
   ~12us); PE warm-up matmuls raise HAM to 8/8 meanwhile.
 - K and V are projected together ([Wk|Wv] packed stationary) with V^T
   transposed to t-layout per slot; the K/V AllGathers fire per half
   (slots 0-1 -> k-pairs 0-7, slots 2-3 -> 8-15) on the gpsimd queue,
   which carries nothing else until the collectives are done.
 - Q is projected as [Wq|Wq] so both PE-array row-group halves get a
   copy of Q^T (S^T matmuls pair kc chunks on disjoint row groups so
   LDWEIGHTS overlaps the other half's matmul).
 - Attention runs in two waves: slots 0-1 over k-pairs 0-7 (needs only
   the first AllGather half), then slots 2-3 over all pairs.  Each
   wave holds 2 PSUM banks for y, leaving 6 for a 3-deep S pipeline.
 - exp on ACT over [128,1024] PSUM tiles (scale folded); causal mask
   is a bf16 DVE multiply on the last 4 pairs of each slot's window.
 - Row-sums ride along as a ones-column in V (65-wide PV stationary);
   per-slot normalization = DVE reciprocal_approx_fast on the [1,512]
   row-sum row + GpSimd partition_broadcast + DVE multiply, so the
   tensor queue never blocks on normalization.
"""

import numpy as np
import ml_dtypes

import concourse.bass as bass
import concourse.bacc as bacc
import concourse.mybir as mybir
from concourse.tile import TileContext
from concourse.tile_rust import add_dep_helper
from concourse.bass_utils import run_bass_kernel_spmd

BF16 = mybir.dt.bfloat16
F32 = mybir.dt.float32
bf16 = ml_dtypes.bfloat16

B, T, C, H = 4, 4096, 1024, 64
N_CORES = 8
TOWN = 2048          # rows owned per core
NSLOT = 4            # q-tiles of 512 rows per core
QT = 512
NKC = 32             # global 128-row k-chunks
NPAIR = 16           # global 256-row k-pair chunks
CC_K = H * TOWN      # K^T shard elements
CC_V = H * TOWN      # V^T shard elements (h-layout)


def build_bass(dbg=False):
    nc = bacc.Bacc(
        "TRN2",
        target_bir_lowering=False,
        debug=False,
        enable_asserts=False,
        num_devices=N_CORES,
    )

    xT = nc.declare_dram_parameter("xT", [C, TOWN], BF16, isOutput=False)
    wkv = nc.declare_dram_parameter("wkv", [C, 128], BF16, isOutput=False)
    wqq = nc.declare_dram_parameter("wqq", [C, 128], BF16, isOutput=False)
    ident = nc.declare_dram_parameter("ident", [128, H], BF16, isOutput=False)
    mask = nc.declare_dram_parameter("mask", [128, 4 * 1024], BF16, isOutput=False)
    out = nc.declare_dram_parameter("out", [H, TOWN], F32, isOutput=True)

    cc_in_k = nc.dram_tensor("cc_in_k", [CC_K], BF16)
    cc_out_k = nc.dram_tensor("cc_out_k", [2 * CC_K], BF16)
    cc_in_v = nc.dram_tensor("cc_in_v", [CC_V], BF16)
    cc_out_v = nc.dram_tensor("cc_out_v", [2 * CC_V], BF16)
    groups = [[2 * i, 2 * i + 1] for i in range(N_CORES // 2)]

    with TileContext(nc) as tc:
        with (
            tc.tile_pool(name="persist", bufs=1) as pp,
            tc.tile_pool(name="work", bufs=3) as wp,
        ):
            # ---- persistent SBUF tensors ----
            xT_sb = pp.tile([128, 8, TOWN], BF16, tag="xT")
            wkv_sb = pp.tile([128, 8, 128], BF16, tag="wkv")
            wqq_sb = pp.tile([128, 8, 128], BF16, tag="wqq")
            id_sb = pp.tile([128, H], BF16, tag="ident")
            mask_sb = pp.tile([128, 4 * 1024], BF16, tag="mask")
            # K^T (rows 0:64) and V^T (rows 64:128) staging for own rows
            kv_stage = pp.tile([128, TOWN], BF16, tag="kvstage")
            # gathered V^T (h-layout) for both ranks, global t order
            vTg = pp.tile([64, T], BF16, tag="vTg")
            qT2 = [
                pp.tile([128, QT], BF16, tag=f"q{s}", name=f"qT2_{s}")
                for s in range(NSLOT)
            ]
            # K^T pair layout: pair p cols p*128..; chunk 2p at
            # partitions 0:64, chunk 2p+1 at 64:128
            kT2 = pp.tile([128, NPAIR * 128], BF16, tag="kT2")
            # V t-layout chunks + ones column (col 64 of each chunk)
            vaug = pp.tile([128, NKC, H + 1], BF16, tag="vaug")
            ones_sb = pp.tile([1, H], F32, tag="ones")
            dume = pp.tile([1, 8], F32, tag="dume")

            # ---- loads; gpsimd queue is reserved for the collectives ----
            nc.vector.memset(ones_sb[:], 1.0)
            nc.vector.memset(vaug[:, :, H : H + 1], 1.0)
            # preload the exp table set while DMAs run
            nc.scalar.activation(
                dume[:], ones_sb[0:1, 0:8], mybir.ActivationFunctionType.Exp
            )
            nc.sync.dma_start(
                out=wkv_sb[:], in_=wkv[:].rearrange("(cc p) m -> p cc m", p=128)
            )
            # xT slot s, cc half h -> 3 queues round-robin; early slots first
            # (gpsimd is clear again before the cc_in writes need it)
            xq = [nc.sync, nc.scalar, nc.gpsimd]
            qi = 0
            for s in range(NSLOT):
                sl = slice(s * QT, (s + 1) * QT)
                for h in range(2):
                    xq[qi % 3].dma_start(
                        out=xT_sb[:, 4 * h : 4 * h + 4, sl],
                        in_=xT[h * 512 : (h + 1) * 512, sl].rearrange(
                            "(cc p) t -> p cc t", p=128
                        ),
                    )
                    qi += 1
            nc.scalar.dma_start(
                out=wqq_sb[:], in_=wqq[:].rearrange("(cc p) m -> p cc m", p=128)
            )
            nc.sync.dma_start(out=id_sb[:], in_=ident[:])
            nc.scalar.dma_start(out=mask_sb[:], in_=mask[:])
            # PE warm-up: scratch matmuls so HAM reaches 8/8 early
            dummy_w = pp.tile([128, 512], BF16, tag="dummyw")
            nc.vector.memset(dummy_w[:], 0.5)

            # ---- projections ----
            with tc.tile_pool(name="proj_ps", bufs=3, space="PSUM") as proj_ps:
                for wi in range(16):
                    wps = proj_ps.tile([128, QT], F32, tag="proj", name="wps")
                    nc.tensor.matmul(
                        wps[:], dummy_w[:, 0:128], dummy_w[:], start=True, stop=True
                    )
                # pass 1: K^T | V^T for own rows; both shards ship to DRAM
                # in h-layout per slot (V transposes happen post-gather,
                # inside the collective dead zone)
                for s in range(NSLOT):
                    sl = slice(s * QT, (s + 1) * QT)
                    ps = proj_ps.tile([128, QT], F32, tag="proj")
                    for cc in range(8):
                        nc.tensor.matmul(
                            ps[:],
                            wkv_sb[:, cc, :],
                            xT_sb[:, cc, sl],
                            start=(cc == 0),
                            stop=(cc == 7),
                        )
                    nc.vector.tensor_copy(kv_stage[:, sl], ps[:])
                    nc.gpsimd.dma_start(
                        out=cc_in_k[s * CC_K // 4 : (s + 1) * CC_K // 4]
                        .rearrange("(p t) -> p t", p=H),
                        in_=kv_stage[0:H, sl],
                    )
                    nc.gpsimd.dma_start(
                        out=cc_in_v[s * CC_V // 4 : (s + 1) * CC_V // 4]
                        .rearrange("(p t) -> p t", p=H),
                        in_=kv_stage[64:128, sl],
                    )
                nc.gpsimd.collective_compute(
                    "AllGather",
                    mybir.AluOpType.bypass,
                    replica_groups=groups,
                    ins=[cc_in_k[:]],
                    outs=[cc_out_k[:]],
                )
                vcc = nc.gpsimd.collective_compute(
                    "AllGather",
                    mybir.AluOpType.bypass,
                    replica_groups=groups,
                    ins=[cc_in_v[:]],
                    outs=[cc_out_v[:]],
                )

                # pass 2: Q^T duplicated to both halves ([Wq|Wq] stationary)
                for s in range(NSLOT):
                    sl = slice(s * QT, (s + 1) * QT)
                    ps = proj_ps.tile([128, QT], F32, tag="proj")
                    for cc in range(8):
                        mmq = nc.tensor.matmul(
                            ps[:],
                            wqq_sb[:, cc, :],
                            xT_sb[:, cc, sl],
                            start=(cc == 0),
                            stop=(cc == 7),
                        )
                        if s == 0 and cc == 0:
                            # keep the scheduler from front-running Q-proj
                            # ahead of the K/V path that feeds the collectives
                            add_dep_helper(
                                mmq.ins, vcc.ins, sync=False, reason="q after cc"
                            )
                    nc.vector.tensor_copy(qT2[s][:], ps[:])

            # ---- readback of gathered K^T and V into compute layouts ----
            # shard gp holds tiles {2s+gp}; kT2 first (gates the S matmuls),
            # vaug on the gpsimd queue (idle once the collectives are done)
            # shard layout is slot-blocked: [gp][s][h=64, t=512]
            ck = cc_out_k[:].rearrange("(gp s h t) -> gp s h t", gp=2, s=NSLOT, h=H)
            cv = cc_out_v[:].rearrange("(gp s h t) -> gp s h t", gp=2, s=NSLOT, h=H)
            for gp in range(2):
                for s in range(NSLOT):
                    g = 2 * s + gp  # global tile
                    ck_s = ck[gp, s].rearrange(
                        "h (chalf hh kk) -> h chalf hh kk", chalf=2, hh=2
                    )
                    for hh in range(2):
                        nc.sync.dma_start(
                            out=kT2[
                                hh * 64 : (hh + 1) * 64,
                                2 * g * 128 : (2 * g + 2) * 128,
                            ].rearrange("h (chalf kk) -> h chalf kk", chalf=2),
                            in_=ck_s[:, :, hh, :],
                        )
            for gp in range(2):
                for s in range(NSLOT):
                    g = 2 * s + gp
                    nc.gpsimd.dma_start(
                        out=vTg[:, g * QT : (g + 1) * QT], in_=cv[gp, s]
                    )

            # V^T -> V t-layout: 32 PE transposes (fills the collective wait)
            with tc.tile_pool(name="vt_ps", bufs=2, space="PSUM") as vt_ps:
                for kc in range(NKC):
                    pst = vt_ps.tile([128, H], BF16, tag="vt")
                    nc.tensor.transpose(
                        pst[:],
                        vTg[:, kc * 128 : (kc + 1) * 128],
                        id_sb[0:64, :],
                    )
                    nc.vector.tensor_copy(vaug[:, kc, 0:H], pst[:])

            # ---- attention: two waves of a pair-of-kc outer flash loop ----
            with (
                tc.tile_pool(name="swide", bufs=3, space="PSUM") as sp,
                tc.tile_pool(name="yacc", bufs=1, space="PSUM") as yp,
            ):

                def norm_slot(s, ytile):
                    # 1/rowsum on the [1,512] sum row, broadcast, scale, out
                    lr = wp.tile([1, QT], F32, tag="lr")
                    nc.vector.tensor_copy(lr[:], ytile[H : H + 1, :])
                    rc = wp.tile([1, QT], F32, tag="rc")
                    nc.vector.reciprocal_approx_fast(rc[:], lr[:])
                    bcb = wp.tile([H, QT], F32, tag="bcb")
                    nc.gpsimd.partition_broadcast(bcb[:], rc[0:1, :])
                    y_sb = wp.tile([H, QT], F32, tag="ysb")
                    nc.vector.tensor_mul(y_sb[:], ytile[0:H, :], bcb[:])
                    nc.scalar.dma_start(
                        out=out[:, s * QT : (s + 1) * QT], in_=y_sb[:]
                    )

                def attn_op(p, s, ytile):
                    sw = sp.tile([128, 1024], F32, tag="swide")
                    nc.tensor.matmul(
                        sw[:, 0:QT],
                        kT2[0:64, p * 128 : (p + 1) * 128],
                        qT2[s][0:64, :],
                        start=True,
                        stop=True,
                    )
                    nc.tensor.matmul(
                        sw[:, QT:1024],
                        kT2[64:128, p * 128 : (p + 1) * 128],
                        qT2[s][64:128, :],
                        start=True,
                        stop=True,
                    )
                    pt = wp.tile([128, 1024], BF16, tag="pt")
                    nc.scalar.activation(
                        pt[:],
                        sw[:],
                        mybir.ActivationFunctionType.Exp,
                        scale=float(H) ** -0.5,
                    )
                    pp_idx = p - 4 * s
                    if pp_idx >= 0:
                        nc.vector.tensor_mul(
                            pt[:],
                            pt[:],
                            mask_sb[:, pp_idx * 1024 : (pp_idx + 1) * 1024],
                        )
                    for half in range(2):
                        kc = 2 * p + half
                        nc.tensor.matmul(
                            ytile[0 : H + 1, :],
                            vaug[:, kc, :],
                            pt[:, half * QT : (half + 1) * QT],
                            start=(p == 0 and half == 0),
                            stop=(p == 4 * s + 3 and half == 1),
                        )
                    if p == 4 * s + 3:
                        norm_slot(s, ytile)

                # warm-keeper matmuls: fill the collective wait after Q-proj
                # so HAM stays at 8/8 into the attention phase
                for wi in range(24):
                    wps2 = sp.tile([128, 1024], F32, tag="swide", name="wps2")
                    wmm = nc.tensor.matmul(
                        wps2[:, 0:QT],
                        dummy_w[:, 0:128],
                        dummy_w[:],
                        start=True,
                        stop=True,
                    )
                    add_dep_helper(
                        wmm.ins, mmq.ins, sync=False, reason="warm after qproj"
                    )

                # wave A: slots 0-1 (k-pairs 0-7)
                ya = [
                    yp.tile([128, QT], F32, tag=f"y{j}", name=f"ya{j}")
                    for j in range(2)
                ]
                for p in range(8):
                    for s in range(2):
                        if p <= 4 * s + 3:
                            attn_op(p, s, ya[s])
                # wave B: slots 2-3 (all k-pairs)
                yb = [
                    yp.tile([128, QT], F32, tag=f"y{j}", name=f"yb{j}")
                    for j in range(2)
                ]
                for p in range(NPAIR):
                    for s in range(2, NSLOT):
                        if p <= 4 * s + 3:
                            attn_op(p, s, yb[s - 2])

    nc.compile()
    return nc


_NC_CACHE = None


def _get_nc():
    global _NC_CACHE
    if _NC_CACHE is None:
        _NC_CACHE = build_bass()
    return _NC_CACHE


def _make_in_maps(x, Wq, Wk, Wv):
    ident = np.zeros((128, H), dtype=bf16)
    ident[0:64, :] = np.eye(H, dtype=bf16)
    ident[64:128, :] = np.eye(H, dtype=bf16)
    wkv = np.concatenate([Wk, Wv], axis=1).astype(bf16)
    wqq = np.concatenate([Wq, Wq], axis=1).astype(bf16)
    # mask sheets [128, 4*1024]: pair-position pp in 0..3, halves of 512
    # keep iff k <= q: p <= f + 512*r - 256*pp - 128*half
    p_idx = np.arange(128)[:, None]
    masks = []
    for r in range(2):
        m = np.zeros((128, 4, 2, QT), dtype=bf16)
        for ppos in range(4):
            for half in range(2):
                f_idx = np.arange(QT)[None, :]
                keep = p_idx <= f_idx + 512 * r - 256 * ppos - 128 * half
                m[:, ppos, half, :] = keep.astype(bf16)
        masks.append(np.ascontiguousarray(m.reshape(128, 4096)))
    in_maps = []
    for c in range(N_CORES):
        b, r = divmod(c, 2)
        rows = np.concatenate(
            [x[b, (2 * s + r) * QT : (2 * s + r + 1) * QT] for s in range(NSLOT)]
        )
        xT_c = np.ascontiguousarray(rows.T).astype(bf16)
        in_maps.append(
            {
                "xT": xT_c,
                "wkv": wkv,
                "wqq": wqq,
                "ident": ident,
                "mask": masks[r],
            }
        )
    return in_maps


def _assemble(results):
    y = np.empty((B, T, H), dtype=np.float32)
    for c in range(N_CORES):
        b, r = divmod(c, 2)
        yt = np.asarray(results[c]["out"], dtype=np.float32).T  # [2048, 64]
        for s in range(NSLOT):
            g = 2 * s + r
            y[b, g * QT : (g + 1) * QT] = yt[s * QT : (s + 1) * QT]
    return y


def run(x, Wq, Wk, Wv, trace=False):
    nc = _get_nc()
    in_maps = _make_in_maps(
        np.asarray(x, np.float32),
        np.asarray(Wq, np.float32),
        np.asarray(Wk, np.float32),
        np.asarray(Wv, np.float32),
    )
    res = run_bass_kernel_spmd(nc, in_maps, core_ids=list(range(N_CORES)), trace=trace)
    return _assemble(res.results), res


def kernel(x, Wq, Wk, Wv):
    y, _ = run(x, Wq, Wk, Wv)
    return y


# revision 37
# speedup vs baseline: 1.1729x; 1.1729x over previous
"""Trainium2 Bass kernel for a causal single-head attention block.

Problem: y = softmax(mask(Q K^T / sqrt(H))) V with
  x  [B=4, T=4096, C=1024] f32,  Wq/Wk/Wv [C, H=64] f32.

Sharding (8 NeuronCores): data-parallel over B across core pairs;
within a pair, T is split by interleaved 512-row tiles (rank r owns
global q-tiles {2s+r}).  Each core projects K/V for its own 2048 rows,
the pair exchanges them via AllGather, and each core runs a
flash-attention style pair-of-kc outer loop over its own query rows.
The graph is identical on all 8 cores (SPMD); all rank-dependent
causality is delivered via input *data* (per-rank mask sheets).

Structure (v5):
 - xT streams in per-slot over the sync/scalar DMA queues (HBM-bound,
   ~12us); PE warm-up matmuls raise HAM to 8/8 meanwhile.
 - K and V are projected together ([Wk|Wv] packed stationary) with V^T
   transposed to t-layout per slot; the K/V AllGathers fire per half
   (slots 0-1 -> k-pairs 0-7, slots 2-3 -> 8-15) on the gpsimd queue,
   which carries nothing else until the collectives are done.
 - Q is projected as [Wq|Wq] so both PE-array row-group halves get a
   copy of Q^T (S^T matmuls pair kc chunks on disjoint row groups so
   LDWEIGHTS overlaps the other half's matmul).
 - Attention runs in two waves: slots 0-1 over k-pairs 0-7 (needs only
   the first AllGather half), then slots 2-3 over all pairs.  Each
   wave holds 2 PSUM banks for y, leaving 6 for a 3-deep S pipeline.
 - exp on ACT over [128,1024] PSUM tiles (scale folded); causal mask
   is a bf16 DVE multiply on the last 4 pairs of each slot's window.
 - Row-sums ride along as a ones-column in V (65-wide PV stationary);
   per-slot normalization = DVE reciprocal_approx_fast on the [1,512]
   row-sum row + GpSimd partition_broadcast + DVE multiply, so the
   tensor queue never blocks on normalization.
"""

import numpy as np
import ml_dtypes

import concourse.bass as bass
import concourse.bacc as bacc
import concourse.mybir as mybir
from concourse.tile import TileContext
from concourse.tile_rust import add_dep_helper
from concourse.bass_utils import run_bass_kernel_spmd

BF16 = mybir.dt.bfloat16
F32 = mybir.dt.float32
bf16 = ml_dtypes.bfloat16

B, T, C, H = 4, 4096, 1024, 64
N_CORES = 8
TOWN = 2048          # rows owned per core
NSLOT = 4            # q-tiles of 512 rows per core
QT = 512
NKC = 32             # global 128-row k-chunks
NPAIR = 16           # global 256-row k-pair chunks
CC_K = H * TOWN      # K^T shard elements
CC_V = 128 * 1024    # V shard elements (t-layout)


def build_bass(dbg=False):
    nc = bacc.Bacc(
        "TRN2",
        target_bir_lowering=False,
        debug=False,
        enable_asserts=False,
        num_devices=N_CORES,
    )

    xT = nc.declare_dram_parameter("xT", [C, TOWN], BF16, isOutput=False)
    wkv = nc.declare_dram_parameter("wkv", [C, 128], BF16, isOutput=False)
    wqq = nc.declare_dram_parameter("wqq", [C, 128], BF16, isOutput=False)
    ident = nc.declare_dram_parameter("ident", [128, H], BF16, isOutput=False)
    mask = nc.declare_dram_parameter("mask", [128, 4 * 1024], BF16, isOutput=False)
    out = nc.declare_dram_parameter("out", [H, TOWN], F32, isOutput=True)

    cc_in_k = nc.dram_tensor("cc_in_k", [CC_K], BF16)
    cc_out_k = [nc.dram_tensor(f"cc_out_k{i}", [CC_K], BF16) for i in range(2)]
    cc_in_v = nc.dram_tensor("cc_in_v", [CC_V], BF16)
    cc_out_v = [nc.dram_tensor(f"cc_out_v{i}", [CC_V], BF16) for i in range(2)]
    groups = [[2 * i, 2 * i + 1] for i in range(N_CORES // 2)]

    with TileContext(nc) as tc:
        with (
            tc.tile_pool(name="persist", bufs=1) as pp,
            tc.tile_pool(name="work", bufs=3) as wp,
        ):
            # ---- persistent SBUF tensors ----
            xT_sb = pp.tile([128, 8, TOWN], BF16, tag="xT")
            wkv_sb = pp.tile([128, 8, 128], BF16, tag="wkv")
            wqq_sb = pp.tile([128, 8, 128], BF16, tag="wqq")
            id_sb = pp.tile([128, H], BF16, tag="ident")
            mask_sb = pp.tile([128, 4 * 1024], BF16, tag="mask")
            # K^T (rows 0:64) and V^T (rows 64:128) staging for own rows
            kv_stage = pp.tile([128, TOWN], BF16, tag="kvstage")
            vstage = pp.tile([128, 1024], BF16, tag="vstage")
            qT2 = [
                pp.tile([128, QT], BF16, tag=f"q{s}", name=f"qT2_{s}")
                for s in range(NSLOT)
            ]
            # K^T pair layout, split by half: pair p in half hf=p//8,
            # cols (p%8)*128..; chunk 2p at partitions 0:64, 2p+1 at 64:128
            kT2 = [
                pp.tile([128, 8 * 128], BF16, tag=f"kT2{i}", name=f"kT2_{i}")
                for i in range(2)
            ]
            # V t-layout chunks + ones column, split by half (16 chunks each)
            vaug = [
                pp.tile([128, 16, H + 1], BF16, tag=f"vaug{i}", name=f"vaug_{i}")
                for i in range(2)
            ]
            ones_sb = pp.tile([1, H], F32, tag="ones")
            dume = pp.tile([1, 8], F32, tag="dume")

            # ---- loads; gpsimd queue is reserved for the collectives ----
            nc.vector.memset(ones_sb[:], 1.0)
            for i in range(2):
                nc.vector.memset(vaug[i][:, :, H : H + 1], 1.0)
            # preload the exp table set while DMAs run
            nc.scalar.activation(
                dume[:], ones_sb[0:1, 0:8], mybir.ActivationFunctionType.Exp
            )
            nc.sync.dma_start(
                out=wkv_sb[:], in_=wkv[:].rearrange("(cc p) m -> p cc m", p=128)
            )
            # xT slot s, cc half h -> sync/scalar queues; slot 0 first
            xq = [nc.sync, nc.scalar]
            for s in range(NSLOT):
                sl = slice(s * QT, (s + 1) * QT)
                for h in range(2):
                    xq[h].dma_start(
                        out=xT_sb[:, 4 * h : 4 * h + 4, sl],
                        in_=xT[h * 512 : (h + 1) * 512, sl].rearrange(
                            "(cc p) t -> p cc t", p=128
                        ),
                    )
            nc.scalar.dma_start(
                out=wqq_sb[:], in_=wqq[:].rearrange("(cc p) m -> p cc m", p=128)
            )
            nc.sync.dma_start(out=id_sb[:], in_=ident[:])
            nc.scalar.dma_start(out=mask_sb[:], in_=mask[:])
            # PE warm-up: scratch matmuls so HAM reaches 8/8 early
            dummy_w = pp.tile([128, 512], BF16, tag="dummyw")
            nc.vector.memset(dummy_w[:], 0.5)

            # ---- projections ----
            with (
                tc.tile_pool(name="proj_ps", bufs=3, space="PSUM") as proj_ps,
                tc.tile_pool(name="vt_ps", bufs=2, space="PSUM") as vt_ps,
            ):
                for wi in range(16):
                    wps = proj_ps.tile([128, QT], F32, tag="proj", name="wps")
                    nc.tensor.matmul(
                        wps[:], dummy_w[:, 0:128], dummy_w[:], start=True, stop=True
                    )
                # pass 1: K^T | V^T for own rows; transposes interleaved
                for s in range(NSLOT):
                    sl = slice(s * QT, (s + 1) * QT)
                    ps = proj_ps.tile([128, QT], F32, tag="proj")
                    for cc in range(8):
                        nc.tensor.matmul(
                            ps[:],
                            wkv_sb[:, cc, :],
                            xT_sb[:, cc, sl],
                            start=(cc == 0),
                            stop=(cc == 7),
                        )
                    nc.vector.tensor_copy(kv_stage[:, sl], ps[:])
                    # V^T -> V (t-layout) for this slot's 4 t-chunks
                    for c in range(4):
                        tcn = 4 * s + c
                        pst = vt_ps.tile([128, H], BF16, tag="vt")
                        nc.tensor.transpose(
                            pst[:],
                            kv_stage[64:128, tcn * 128 : (tcn + 1) * 128],
                            id_sb[64:128, :],
                        )
                        nc.vector.tensor_copy(
                            vstage[:, tcn * H : (tcn + 1) * H], pst[:]
                        )
                    if s % 2 == 1:
                        hf = s // 2
                        hsl = slice(hf * 1024, (hf + 1) * 1024)
                        nc.gpsimd.dma_start(
                            out=cc_in_k[hf * CC_K // 2 : (hf + 1) * CC_K // 2]
                            .rearrange("(p t) -> p t", p=H),
                            in_=kv_stage[0:H, hsl],
                        )
                        nc.gpsimd.collective_compute(
                            "AllGather",
                            mybir.AluOpType.bypass,
                            replica_groups=groups,
                            ins=[cc_in_k[hf * CC_K // 2 : (hf + 1) * CC_K // 2]],
                            outs=[cc_out_k[hf][:]],
                        )
                        nc.gpsimd.dma_start(
                            out=cc_in_v[hf * CC_V // 2 : (hf + 1) * CC_V // 2]
                            .rearrange("(p c) -> p c", p=128),
                            in_=vstage[:, hf * 512 : (hf + 1) * 512],
                        )
                        vcc = nc.gpsimd.collective_compute(
                            "AllGather",
                            mybir.AluOpType.bypass,
                            replica_groups=groups,
                            ins=[cc_in_v[hf * CC_V // 2 : (hf + 1) * CC_V // 2]],
                            outs=[cc_out_v[hf][:]],
                        )

                # pass 2: Q^T duplicated to both halves ([Wq|Wq] stationary)
                for s in range(NSLOT):
                    sl = slice(s * QT, (s + 1) * QT)
                    ps = proj_ps.tile([128, QT], F32, tag="proj")
                    for cc in range(8):
                        mmq = nc.tensor.matmul(
                            ps[:],
                            wqq_sb[:, cc, :],
                            xT_sb[:, cc, sl],
                            start=(cc == 0),
                            stop=(cc == 7),
                        )
                        if s == 0 and cc == 0:
                            # keep the scheduler from front-running Q-proj
                            # ahead of the K/V path that feeds the collectives
                            add_dep_helper(
                                mmq.ins, vcc.ins, sync=False, reason="q after cc"
                            )
                    nc.vector.tensor_copy(qT2[s][:], ps[:])

            # ---- readback of gathered K^T and V into compute layouts ----
            # AG half hf covers slots 0-1/2-3 of both ranks = tiles 4hf..4hf+3
            for hf in range(2):
                ck = cc_out_k[hf][:].rearrange("(gp h sc) -> gp h sc", gp=2, h=H)
                cv = cc_out_v[hf][:].rearrange("(gp p sc) -> gp p sc", gp=2, p=128)
                for gp in range(2):
                    for s2 in range(2):
                        g = 2 * (2 * hf + s2) + gp  # global tile
                        lp = 2 * g - 8 * hf  # local pair index in half
                        ck_s = ck[gp, :, s2 * QT : (s2 + 1) * QT].rearrange(
                            "h (chalf hh kk) -> h chalf hh kk", chalf=2, hh=2
                        )
                        for hh in range(2):
                            nc.sync.dma_start(
                                out=kT2[hf][
                                    hh * 64 : (hh + 1) * 64,
                                    lp * 128 : (lp + 2) * 128,
                                ].rearrange("h (chalf kk) -> h chalf kk", chalf=2),
                                in_=ck_s[:, :, hh, :],
                            )
                        nc.sync.dma_start(
                            out=vaug[hf][:, 4 * g - 16 * hf : 4 * g - 16 * hf + 4, 0:H],
                            in_=cv[gp, :, s2 * 256 : (s2 + 1) * 256].rearrange(
                                "p (c h) -> p c h", h=H
                            ),
                        )

            # ---- attention: two waves of a pair-of-kc outer flash loop ----
            with (
                tc.tile_pool(name="swide", bufs=3, space="PSUM") as sp,
                tc.tile_pool(name="yacc", bufs=1, space="PSUM") as yp,
            ):

                def norm_slot(s, ytile):
                    # 1/rowsum on the [1,512] sum row, broadcast, scale, out
                    lr = wp.tile([1, QT], F32, tag="lr")
                    nc.vector.tensor_copy(lr[:], ytile[H : H + 1, :])
                    rc = wp.tile([1, QT], F32, tag="rc")
                    nc.vector.reciprocal_approx_fast(rc[:], lr[:])
                    bcb = wp.tile([H, QT], F32, tag="bcb")
                    nc.gpsimd.partition_broadcast(bcb[:], rc[0:1, :])
                    y_sb = wp.tile([H, QT], F32, tag="ysb")
                    nc.vector.tensor_mul(y_sb[:], ytile[0:H, :], bcb[:])
                    nc.scalar.dma_start(
                        out=out[:, s * QT : (s + 1) * QT], in_=y_sb[:]
                    )

                def attn_op(p, s, ytile):
                    hf, lp = p // 8, p % 8
                    sw = sp.tile([128, 1024], F32, tag="swide")
                    nc.tensor.matmul(
                        sw[:, 0:QT],
                        kT2[hf][0:64, lp * 128 : (lp + 1) * 128],
                        qT2[s][0:64, :],
                        start=True,
                        stop=True,
                    )
                    nc.tensor.matmul(
                        sw[:, QT:1024],
                        kT2[hf][64:128, lp * 128 : (lp + 1) * 128],
                        qT2[s][64:128, :],
                        start=True,
                        stop=True,
                    )
                    pt = wp.tile([128, 1024], BF16, tag="pt")
                    nc.scalar.activation(
                        pt[:],
                        sw[:],
                        mybir.ActivationFunctionType.Exp,
                        scale=float(H) ** -0.5,
                    )
                    pp_idx = p - 4 * s
                    if pp_idx >= 0:
                        nc.vector.tensor_mul(
                            pt[:],
                            pt[:],
                            mask_sb[:, pp_idx * 1024 : (pp_idx + 1) * 1024],
                        )
                    for half in range(2):
                        kc = (2 * p + half) % 16
                        nc.tensor.matmul(
                            ytile[0 : H + 1, :],
                            vaug[hf][:, kc, :],
                            pt[:, half * QT : (half + 1) * QT],
                            start=(p == 0 and half == 0),
                            stop=(p == 4 * s + 3 and half == 1),
                        )
                    if p == 4 * s + 3:
                        norm_slot(s, ytile)

                # warm-keeper matmuls: fill the collective wait after Q-proj
                # so HAM stays at 8/8 into the attention phase
                for wi in range(48):
                    wps2 = sp.tile([128, 1024], F32, tag="swide", name="wps2")
                    wmm = nc.tensor.matmul(
                        wps2[:, 0:256],
                        dummy_w[:, 0:128],
                        dummy_w[:, 0:256],
                        start=True,
                        stop=True,
                    )
                    add_dep_helper(
                        wmm.ins, mmq.ins, sync=False, reason="warm after qproj"
                    )

                # wave A: slots 0-1 (k-pairs 0-7, first AG half only)
                ya = [
                    yp.tile([128, QT], F32, tag=f"y{j}", name=f"ya{j}")
                    for j in range(2)
                ]
                for p in range(8):
                    for s in range(2):
                        if p <= 4 * s + 3:
                            attn_op(p, s, ya[s])
                # wave B: slots 2-3 (all k-pairs)
                yb = [
                    yp.tile([128, QT], F32, tag=f"y{j}", name=f"yb{j}")
                    for j in range(2)
                ]
                for p in range(NPAIR):
                    for s in range(2, NSLOT):
                        if p <= 4 * s + 3:
                            attn_op(p, s, yb[s - 2])

    nc.compile()
    return nc


_NC_CACHE = None


def _get_nc():
    global _NC_CACHE
    if _NC_CACHE is None:
        _NC_CACHE = build_bass()
    return _NC_CACHE


def _make_in_maps(x, Wq, Wk, Wv):
    ident = np.zeros((128, H), dtype=bf16)
    ident[64:128, :] = np.eye(H, dtype=bf16)
    wkv = np.concatenate([Wk, Wv], axis=1).astype(bf16)
    wqq = np.concatenate([Wq, Wq], axis=1).astype(bf16)
    # mask sheets [128, 4*1024]: pair-position pp in 0..3, halves of 512
    # keep iff k <= q: p <= f + 512*r - 256*pp - 128*half
    p_idx = np.arange(128)[:, None]
    masks = []
    for r in range(2):
        m = np.zeros((128, 4, 2, QT), dtype=bf16)
        for ppos in range(4):
            for half in range(2):
                f_idx = np.arange(QT)[None, :]
                keep = p_idx <= f_idx + 512 * r - 256 * ppos - 128 * half
                m[:, ppos, half, :] = keep.astype(bf16)
        masks.append(np.ascontiguousarray(m.reshape(128, 4096)))
    in_maps = []
    for c in range(N_CORES):
        b, r = divmod(c, 2)
        rows = np.concatenate(
            [x[b, (2 * s + r) * QT : (2 * s + r + 1) * QT] for s in range(NSLOT)]
        )
        xT_c = np.ascontiguousarray(rows.T).astype(bf16)
        in_maps.append(
            {
                "xT": xT_c,
                "wkv": wkv,
                "wqq": wqq,
                "ident": ident,
                "mask": masks[r],
            }
        )
    return in_maps


def _assemble(results):
    y = np.empty((B, T, H), dtype=np.float32)
    for c in range(N_CORES):
        b, r = divmod(c, 2)
        yt = np.asarray(results[c]["out"], dtype=np.float32).T  # [2048, 64]
        for s in range(NSLOT):
            g = 2 * s + r
            y[b, g * QT : (g + 1) * QT] = yt[s * QT : (s + 1) * QT]
    return y


def run(x, Wq, Wk, Wv, trace=False):
    nc = _get_nc()
    in_maps = _make_in_maps(
        np.asarray(x, np.float32),
        np.asarray(Wq, np.float32),
        np.asarray(Wk, np.float32),
        np.asarray(Wv, np.float32),
    )
    res = run_bass_kernel_spmd(nc, in_maps, core_ids=list(range(N_CORES)), trace=trace)
    return _assemble(res.results), res


def kernel(x, Wq, Wk, Wv):
    y, _ = run(x, Wq, Wk, Wv)
    return y
